# revision 1
# baseline (speedup 1.0000x reference)
"""Trainium2 Bass kernel for 4D conv (VALID, stride 1, channels-last).

x: [4, 20, 20, 40, 40, 8] f32, kernel: [3, 3, 3, 3, 8, 16], bias: [...,16]
out: [4, 18, 18, 38, 38, 16] f32

Strategy (8 NeuronCores, SPMD):
  - Shard (batch 4) x (T-halves 2) -> 8 shards. Each core gets
    x[b, 9*th : 9*th+11] (halo 2 in T) and computes out[b, 9*th : 9*th+9].
  - On-chip: per T-plane, load natural (z,h)-row-major chunks, PE-transpose
    to (w,c)-major tiles X_T[(w-wstart)*8+c, z*40+h] for 5 overlapping
    w-blocks (w starts 0,8,16,24,32).
  - Conv as Toeplitz-banded matmul: lhsT[(wl,c), (w'l,co)] holds
    k[dt,dz,dh, wl-w'l, c, co] (band 0<=wl-w'l<=2), contracting K=(10w x 8c)
    =80 rows; M=128=(8 w' x 16 co); 27 taps (dt,dz,dh) accumulate in PSUM
    with free-dim shifts dz*40+dh into X_T columns; N=342=(9 z' x 38 h').
  - Bias added during PSUM->SBUF evacuation (per-partition scalar), output
    written in a blocked layout [t',zh,wb,128,342]; host rearranges.
"""

import sys

if "/opt/trn_rl_repo" not in sys.path:
    sys.path.insert(0, "/opt/trn_rl_repo")

from contextlib import ExitStack

import ml_dtypes
import numpy as np

import concourse.bass as bass
import concourse.tile as tile
from concourse import bacc, mybir
from concourse.bass_utils import run_bass_kernel_spmd
from concourse.masks import make_identity

F32 = mybir.dt.float32
BF16 = mybir.dt.bfloat16

# Problem geometry (hardcoded)
B, T, Z, H, W, CIN = 4, 20, 20, 40, 40, 8
KT = KZ = KH = KW = 3
COUT = 16
TP = 9  # output t' per core (T' = 18 split across 2 cores)
TL = TP + KT - 1  # input t planes per core = 11
ZP, HP, WP = Z - 2, H - 2, W - 2  # 18, 38, 38
NTAP = KT * KZ * KH  # 27 taps accumulated in PSUM
WBLOCKS = 5  # w starts 0,8,16,24,32
NCOLS = 9 * HP  # 342 columns per matmul (9 z' x 38 h')

# transpose free-slices of the (w,c)=320 row: (offset, width) per w-block
_TR_SLICES = [(0, 128), (64, 128), (128, 128), (192, 128), (256, 64)]

LAST_RESULTS = None  # BassKernelResults of the most recent run (for test.py)
REPS = 1  # >1 wraps the body in a hardware loop (timing experiments only)


def _build_program():
    nc = bacc.Bacc("TRN2", target_bir_lowering=False, debug=False, num_devices=8)

    x_d = nc.dram_tensor("x", [TL, Z, H, W, CIN], F32, kind="ExternalInput").ap()
    wt_d = nc.dram_tensor("wt", [NTAP, 128, 128], BF16, kind="ExternalInput").ap()
    bias_d = nc.dram_tensor("bias128", [128, 1], F32, kind="ExternalInput").ap()
    out_d = nc.dram_tensor(
        "out", [TP, 2, WBLOCKS, 128, NCOLS], F32, kind="ExternalOutput"
    ).ap()

    with ExitStack() as ctx:
        tc = ctx.enter_context(tile.TileContext(nc))
        consts = ctx.enter_context(tc.tile_pool(name="consts", bufs=1))
        xt_pool = ctx.enter_context(tc.tile_pool(name="xt", bufs=4))
        chunk_pool = ctx.enter_context(tc.tile_pool(name="chunk", bufs=3))
        tpsum_pool = ctx.enter_context(tc.tile_pool(name="tpsum", bufs=3, space="PSUM"))
        mpsum_pool = ctx.enter_context(tc.tile_pool(name="mpsum", bufs=4, space="PSUM"))
        outp_pool = ctx.enter_context(tc.tile_pool(name="outp", bufs=4))

        ident = consts.tile([128, 128], F32)
        make_identity(nc, ident)

        wt_s = consts.tile([128, NTAP * 128], BF16)
        for j in range(NTAP):
            nc.sync.dma_start(wt_s[:, j * 128 : (j + 1) * 128], wt_d[j])

        bias_s = consts.tile([128, 1], F32)
        nc.sync.dma_start(bias_s[:, :], bias_d[:, :])

        planes = {}

        def prep_plane(t):
            tiles = [
                xt_pool.tile([128, Z * H], BF16, tag=f"xt{i}", name=f"xt{i}_{t}")
                for i in range(WBLOCKS)
            ]
            x_t = x_d[t].rearrange("z h w c -> (z h) (w c)")  # [800, 320]
            nrows = Z * H
            for r0 in range(0, nrows, 128):
                r = min(128, nrows - r0)
                ch = chunk_pool.tile([128, W * CIN], F32, tag="chunk")
                nc.sync.dma_start(ch[:r, :], x_t[r0 : r0 + r, :])
                for bi, (f0, fw) in enumerate(_TR_SLICES):
                    ps = tpsum_pool.tile([128, 128], F32, tag="tps")
                    nc.tensor.transpose(ps[:fw, :r], ch[:r, f0 : f0 + fw], ident[:r, :r])
                    # f32 PSUM -> bf16 SBUF cast during evacuation
                    nc.vector.tensor_copy(tiles[bi][:fw, r0 : r0 + r], ps[:fw, :r])
            planes[t] = tiles

        def do_tp(tp):
            for zh in range(2):
                for wb in range(WBLOCKS):
                    k = 64 if wb == WBLOCKS - 1 else 80
                    ps = mpsum_pool.tile([128, NCOLS], F32, tag="mps")
                    for j in range(NTAP):
                        dt_, r = divmod(j, KZ * KH)
                        dz, dh = divmod(r, KH)
                        v = planes[tp + dt_][wb].rearrange("p (z h) -> p z h", h=H)
                        rhs = v[0:k, zh * 9 + dz : zh * 9 + dz + 9, dh : dh + HP]
                        lhsT = wt_s[0:k, j * 128 : (j + 1) * 128]
                        nc.tensor.matmul(
                            ps[:, :], lhsT, rhs, start=(j == 0), stop=(j == NTAP - 1)
                        )
                    ot = outp_pool.tile([128, NCOLS], F32, tag="out")
                    nc.vector.tensor_scalar_add(ot[:, :], ps[:, :], bias_s[:, 0:1])
                    nc.sync.dma_start(out_d[tp, zh, wb], ot[:, :])

        def body():
            planes.clear()
            for t in range(KT):
                prep_plane(t)
            for tp in range(TP):
                do_tp(tp)
                if tp + KT < TL:
                    prep_plane(tp + KT)

        if REPS > 1:
            with tc.For_i(0, REPS, 1):
                body()
        else:
            body()

    nc.compile()
    return nc


def _host_weights(kern):
    """Toeplitz-banded weight matrices, one per (dt,dz,dh) tap."""
    wt = np.zeros((NTAP, 128, 128), np.float32)
    for dt_ in range(KT):
        for dz in range(KZ):
            for dh in range(KH):
                j = (dt_ * KZ + dz) * KH + dh
                for dw in range(KW):
                    for wpl in range(8):
                        wl = wpl + dw
                        wt[
                            j,
                            wl * CIN : (wl + 1) * CIN,
                            wpl * COUT : (wpl + 1) * COUT,
                        ] = kern[dt_, dz, dh, dw]
    return wt.astype(ml_dtypes.bfloat16)


def _core_inputs(x, kern, bias, core):
    wt = _host_weights(kern)
    bias128 = np.tile(bias, 8).reshape(128, 1).astype(np.float32)
    b, th = divmod(core, 2)
    return {
        "x": np.ascontiguousarray(x[b, 9 * th : 9 * th + TL]),
        "wt": wt,
        "bias128": bias128,
    }


def _assemble_core_output(outs, core):
    a = outs["out"].reshape(TP, 2, WBLOCKS, 8, COUT, 9, HP)
    a = a.transpose(0, 1, 5, 6, 2, 3, 4).reshape(TP, ZP, HP, 40, COUT)
    return a[:, :, :, :WP, :]


def _expected_core_output(expected, core):
    b, th = divmod(core, 2)
    return expected[b, 9 * th : 9 * th + TP]


def kernel(x, kernel, bias):
    global LAST_RESULTS
    x = np.asarray(x, np.float32)
    kern = np.asarray(kernel, np.float32)
    bias = np.asarray(bias, np.float32).reshape(COUT)

    nc = _build_program()

    core_ids = list(range(8))
    in_maps = [_core_inputs(x, kern, bias, core) for core in core_ids]

    res = run_bass_kernel_spmd(nc, in_maps, core_ids)
    LAST_RESULTS = res

    out = np.empty((B, 2 * TP, ZP, HP, WP, COUT), np.float32)
    for core in core_ids:
        b, th = divmod(core, 2)
        out[b, 9 * th : 9 * th + TP] = _assemble_core_output(res.results[core], core)
    return out



# revision 3
# speedup vs baseline: 91.6936x; 91.6936x over previous
"""Trainium2 Bass kernel for 4D conv (VALID, stride 1, channels-last).

x: [4, 20, 20, 40, 40, 8] f32, kernel: [3, 3, 3, 3, 8, 16], bias: [...,16]
out: [4, 18, 18, 38, 38, 16] f32

Polyphase / space-to-depth scheme (8 NeuronCores, SPMD):
  - Shard (batch 4) x (T-halves 2) -> 8 shards; core computes out t' 9-slab.
  - Outputs tiled in 2x2x2 blocks over (z',h',w'): M = 128 = (2z' 2h' 2w' 16co).
  - Contraction K = 128 = (4zl x 4hl x 4wl x 2cc): the full 4x4x4 input
    window of a 2x2x2 output block, for a 2-channel group. All three
    spatial kernel dims are folded into K; PSUM accumulates only
    12 taps = (3 dt) x (4 channel-groups).
  - N = 361 = the 19x19 (h0,w0) block grid (exact, no padding).
  - Host pre-packs x into x4[z0, (g, rows, t, h0, w0)] bf16 (8x window
    duplication; ~36.6 MB per core) and the 12 banded weight matrices.
    Output written bf16 [z0, 128, (tp, h0, w0)]; host rearranges+casts.
  - One DMA in per z-slab (4.07 MB), one DMA out per z-slab (832 KB).
  PE: 972 matmuls x 361 cycles ~= 146 us; ~44 MB DMA ~= 134 us (overlapped).
"""

import sys

if "/opt/trn_rl_repo" not in sys.path:
    sys.path.insert(0, "/opt/trn_rl_repo")

from contextlib import ExitStack

import ml_dtypes
import numpy as np

import concourse.bass as bass
import concourse.tile as tile
from concourse import bacc, mybir
from concourse.bass_utils import run_bass_kernel_spmd

F32 = mybir.dt.float32
BF16 = mybir.dt.bfloat16

# Problem geometry (hardcoded)
B, T, Z, H, W, CIN = 4, 20, 20, 40, 40, 8
KT = KZ = KH = KW = 3
COUT = 16
TP = 9  # output t' per core (T' = 18 split across 2 cores)
TL = TP + KT - 1  # input t planes per core = 11
ZP, HP, WP = Z - 2, H - 2, W - 2  # 18, 38, 38
NZ0 = ZP // 2  # 9 z-block origins (z0 = 2*z0i)
NG = 19  # h0/w0 block grid (2*19 = 38 outputs, exact)
NCOLS = NG * NG  # 361 columns per matmul
NTAP = KT * 4  # 12 taps: (dt, channel-group)
KROWS = 128  # (zl4, hl4, wl4, cc2)

LAST_RESULTS = None
REPS = 1


def _build_program():
    nc = bacc.Bacc("TRN2", target_bir_lowering=False, debug=False, num_devices=8)

    x4_d = nc.dram_tensor(
        "x4", [NZ0, 4, KROWS, TL, NCOLS], BF16, kind="ExternalInput"
    ).ap()
    wt_d = nc.dram_tensor("wt", [NTAP, KROWS, 128], BF16, kind="ExternalInput").ap()
    bias_d = nc.dram_tensor("bias128", [128, 1], F32, kind="ExternalInput").ap()
    out_d = nc.dram_tensor(
        "out", [NZ0, 128, TP, NCOLS], BF16, kind="ExternalOutput"
    ).ap()

    with ExitStack() as ctx:
        tc = ctx.enter_context(tile.TileContext(nc))
        consts = ctx.enter_context(tc.tile_pool(name="consts", bufs=1))
        x_pool = ctx.enter_context(tc.tile_pool(name="xp", bufs=2))
        mpsum_pool = ctx.enter_context(tc.tile_pool(name="mpsum", bufs=6, space="PSUM"))
        outp_pool = ctx.enter_context(tc.tile_pool(name="outp", bufs=2))

        wt_s = consts.tile([KROWS, NTAP * 128], BF16)
        wt_sv = wt_s.rearrange("p (j m) -> p j m", j=NTAP)
        wt_dv = wt_d.rearrange("j p m -> p j m")
        # first two taps land fast so matmuls can start ~2us in
        nc.scalar.dma_start(wt_sv[:, 0:2], wt_dv[:, 0:2])
        nc.scalar.dma_start(wt_sv[:, 2:NTAP], wt_dv[:, 2:NTAP])

        bias_s = consts.tile([128, 1], F32)
        nc.scalar.dma_start(bias_s[:, :], bias_d[:, :])

        slabs = {}

        def prep_slab(z0i, split_first=False):
            tl = x_pool.tile([KROWS, 4 * TL * NCOLS], BF16, tag="x", name=f"x_{z0i}")
            fsz = TL * NCOLS
            if split_first:
                # fetch in consumption order: t-planes 0-2 of each group
                # first so tp=0 matmuls can start ~3us in
                c3 = 3 * NCOLS
                for g in range(4):
                    nc.sync.dma_start(
                        tl[:, g * fsz : g * fsz + c3], x4_d[z0i, g, :, 0:3]
                    )
                for g in range(4):
                    nc.sync.dma_start(
                        tl[:, g * fsz + c3 : (g + 1) * fsz], x4_d[z0i, g, :, 3:TL]
                    )
            else:
                for g in range(4):
                    nc.sync.dma_start(tl[:, g * fsz : (g + 1) * fsz], x4_d[z0i, g])
            slabs[z0i] = tl

        def do_slab(z0i, split_out=False):
            v = slabs[z0i].rearrange("p (g t n) -> p g t n", g=4, t=TL)
            ob = outp_pool.tile([128, TP * NCOLS], BF16, tag="out")
            obv = ob.rearrange("p (t n) -> p t n", t=TP)
            for tp in range(TP):
                ps = mpsum_pool.tile([128, NCOLS], F32, tag="mps")
                for j in range(NTAP):
                    dt_, g = divmod(j, 4)
                    rhs = v[0:KROWS, g, tp + dt_]
                    lhsT = wt_s[0:KROWS, j * 128 : (j + 1) * 128]
                    nc.tensor.matmul(
                        ps[:, :], lhsT, rhs, start=(j == 0), stop=(j == NTAP - 1)
                    )
                nc.vector.tensor_scalar_add(
                    ob[:, tp * NCOLS : (tp + 1) * NCOLS], ps[:, :], bias_s[:, 0:1]
                )
                if split_out and tp % 3 == 2:
                    # drain the tail: ship each completed third immediately
                    nc.scalar.dma_start(
                        out_d[z0i, :, tp - 2 : tp + 1], obv[:, tp - 2 : tp + 1]
                    )
            if not split_out:
                nc.scalar.dma_start(out_d[z0i], obv)

        def body():
            slabs.clear()
            prep_slab(0, split_first=True)
            for z0i in range(NZ0):
                if z0i + 1 < NZ0:
                    prep_slab(z0i + 1)
                do_slab(z0i, split_out=(z0i == NZ0 - 1))
                del slabs[z0i]

        if REPS > 1:
            with tc.For_i(0, REPS, 1):
                body()
        else:
            body()

    nc.compile()
    return nc


def _host_weights(kern):
    """wt[(dt,g), (zl,hl,wl,cc), (z',h',w',co)] banded block weights."""
    wt = np.zeros((KT, 4, 4, 4, 4, 2, 2, 2, 2, COUT), np.float32)
    # idx: [dt, g, zl, hl, wl, cc, z', h', w', co]
    kr = kern.reshape(KT, KZ, KH, KW, 4, 2, COUT)  # c = 2g + cc
    for dz in range(KZ):
        for dh in range(KH):
            for dw in range(KW):
                for zp in range(2):
                    for hp in range(2):
                        for wp in range(2):
                            wt[:, :, zp + dz, hp + dh, wp + dw, :, zp, hp, wp, :] = kr[
                                :, dz, dh, dw
                            ]
    return wt.reshape(NTAP, KROWS, 128).astype(ml_dtypes.bfloat16)


def _host_x4(xc):
    """Pack a core's x slice [TL,Z,H,W,CIN] f32 into
    x4[z0i, g, (zl hl wl cc), (t, h0, w0)] bf16."""
    xb = xc.astype(ml_dtypes.bfloat16).astype(np.float32)
    # windows[t, z0, h0, w0, c, zl, hl, wl] with stride-2 origins
    win = np.lib.stride_tricks.sliding_window_view(xb, (4, 4, 4), axis=(1, 2, 3))
    win = win[:, ::2, ::2, ::2]  # [TL, 9, 19, 19, 8, 4, 4, 4]
    # -> [z0, zl, hl, wl, c, t, h0, w0]
    win = win.transpose(1, 5, 6, 7, 4, 0, 2, 3)
    # split c -> (g, cc) and move g in front of the row dims
    win = win.reshape(NZ0, 4, 4, 4, 4, 2, TL, NG, NG).transpose(
        0, 4, 1, 2, 3, 5, 6, 7, 8
    )
    x4 = win.reshape(NZ0, 4, KROWS, TL * NCOLS)
    return np.ascontiguousarray(x4).astype(ml_dtypes.bfloat16)


_WT_CACHE = {}


def _core_inputs(x, kern, bias, core):
    key = kern.tobytes()
    if _WT_CACHE.get("key") != key:
        _WT_CACHE["key"] = key
        _WT_CACHE["wt"] = _host_weights(kern)
    wt = _WT_CACHE["wt"]
    bias128 = np.tile(bias, 8).reshape(128, 1).astype(np.float32)
    b, th = divmod(core, 2)
    return {
        "x4": _host_x4(x[b, 9 * th : 9 * th + TL]),
        "wt": wt,
        "bias128": bias128,
    }


def _assemble_core_output(outs, core):
    # out [NZ0, (2z' 2h' 2w' 16co), TP, (19h0, 19w0)] bf16
    a = np.asarray(outs["out"]).reshape(NZ0, 2, 2, 2, COUT, TP, NG, NG)
    # -> [t', (z0 z'), (h0 h'), (w0 w'), co]
    a = a.transpose(5, 0, 1, 6, 2, 7, 3, 4)
    a = a.reshape(TP, ZP, HP, WP, COUT)
    return a.astype(np.float32)


def kernel(x, kernel, bias):
    global LAST_RESULTS
    x = np.asarray(x, np.float32)
    kern = np.asarray(kernel, np.float32)
    bias = np.asarray(bias, np.float32).reshape(COUT)

    nc = _build_program()

    core_ids = list(range(8))
    in_maps = [_core_inputs(x, kern, bias, core) for core in core_ids]

    res = run_bass_kernel_spmd(nc, in_maps, core_ids)
    LAST_RESULTS = res

    out = np.empty((B, 2 * TP, ZP, HP, WP, COUT), np.float32)
    for core in core_ids:
        b, th = divmod(core, 2)
        out[b, 9 * th : 9 * th + TP] = _assemble_core_output(res.results[core], core)
    return out


# revision 4
# speedup vs baseline: 109.7602x; 1.1970x over previous
"""Trainium2 Bass kernel for 4D conv (VALID, stride 1, channels-last).

x: [4, 20, 20, 40, 40, 8] f32, kernel: [3, 3, 3, 3, 8, 16], bias: [...,16]
out: [4, 18, 18, 38, 38, 16] f32

Polyphase / space-to-depth scheme (8 NeuronCores, SPMD):
  - Shard (batch 4) x (T-halves 2) -> 8 shards; core computes out t' 9-slab.
  - Outputs tiled in 2x2x2 blocks over (z',h',w'): M = 128 = (2z' 2h' 2w' 16co).
  - Contraction K = 128 = (4zl x 4hl x 4wl x 2cc): the full 4x4x4 input
    window of a 2x2x2 output block, for a 2-channel group. All three
    spatial kernel dims are folded into K; PSUM accumulates only
    12 taps = (3 dt) x (4 channel-groups).
  - N = 361 = the 19x19 (h0,w0) block grid (exact, no padding).
  - Host pre-packs x into x4[z0, (g, rows, t, h0, w0)] bf16 (8x window
    duplication; ~36.6 MB per core) and the 12 banded weight matrices.
    Output written bf16 [z0, 128, (tp, h0, w0)]; host rearranges+casts.
  - One DMA in per z-slab (4.07 MB), one DMA out per z-slab (832 KB).
  PE: 972 matmuls x 361 cycles ~= 146 us; ~44 MB DMA ~= 134 us (overlapped).
"""

import sys

if "/opt/trn_rl_repo" not in sys.path:
    sys.path.insert(0, "/opt/trn_rl_repo")

from contextlib import ExitStack

import ml_dtypes
import numpy as np

import concourse.bass as bass
import concourse.tile as tile
from concourse import bacc, mybir
from concourse.bass_utils import run_bass_kernel_spmd

F32 = mybir.dt.float32
BF16 = mybir.dt.bfloat16

# Problem geometry (hardcoded)
B, T, Z, H, W, CIN = 4, 20, 20, 40, 40, 8
KT = KZ = KH = KW = 3
COUT = 16
TP = 9  # output t' per core (T' = 18 split across 2 cores)
TL = TP + KT - 1  # input t planes per core = 11
ZP, HP, WP = Z - 2, H - 2, W - 2  # 18, 38, 38
NZ0 = ZP // 2  # 9 z-block origins (z0 = 2*z0i)
NG = 19  # h0/w0 block grid (2*19 = 38 outputs, exact)
NCOLS = NG * NG  # 361 columns per matmul
NTAP = KT * 4  # 12 taps: (dt, channel-group)
KROWS = 128  # (zl4, hl4, wl4, cc2)

LAST_RESULTS = None
REPS = 1


def _build_program():
    nc = bacc.Bacc("TRN2", target_bir_lowering=False, debug=False, num_devices=8)

    x4_d = nc.dram_tensor(
        "x4", [NZ0, 4, KROWS, TL, NCOLS], BF16, kind="ExternalInput"
    ).ap()
    wt_d = nc.dram_tensor("wt", [NTAP, KROWS, 128], BF16, kind="ExternalInput").ap()
    bias_d = nc.dram_tensor("bias128", [128, 1], F32, kind="ExternalInput").ap()
    out_d = nc.dram_tensor(
        "out", [NZ0, 128, TP, NCOLS], BF16, kind="ExternalOutput"
    ).ap()

    with ExitStack() as ctx:
        tc = ctx.enter_context(tile.TileContext(nc))
        consts = ctx.enter_context(tc.tile_pool(name="consts", bufs=1))
        x_pool = ctx.enter_context(tc.tile_pool(name="xp", bufs=3))
        mpsum_pool = ctx.enter_context(tc.tile_pool(name="mpsum", bufs=6, space="PSUM"))
        outp_pool = ctx.enter_context(tc.tile_pool(name="outp", bufs=2))

        wt_s = consts.tile([KROWS, NTAP * 128], BF16)
        wt_sv = wt_s.rearrange("p (j m) -> p j m", j=NTAP)
        wt_dv = wt_d.rearrange("j p m -> p j m")
        # first two taps land fast so matmuls can start ~2us in
        nc.scalar.dma_start(wt_sv[:, 0:2], wt_dv[:, 0:2])
        nc.scalar.dma_start(wt_sv[:, 2:NTAP], wt_dv[:, 2:NTAP])

        bias_s = consts.tile([128, 1], F32)
        nc.scalar.dma_start(bias_s[:, :], bias_d[:, :])

        slabs = {}

        def prep_slab(z0i, split_first=False):
            tl = x_pool.tile([KROWS, 4 * TL * NCOLS], BF16, tag="x", name=f"x_{z0i}")
            fsz = TL * NCOLS
            if split_first:
                # fetch in consumption order: t-planes 0-2 of each group
                # first so tp=0 matmuls can start ~3us in
                c3 = 3 * NCOLS
                for g in range(4):
                    nc.sync.dma_start(
                        tl[:, g * fsz : g * fsz + c3], x4_d[z0i, g, :, 0:3]
                    )
                for g in range(4):
                    nc.sync.dma_start(
                        tl[:, g * fsz + c3 : (g + 1) * fsz], x4_d[z0i, g, :, 3:TL]
                    )
            else:
                eng = nc.sync if z0i % 2 == 0 else nc.scalar
                for g in range(4):
                    eng.dma_start(tl[:, g * fsz : (g + 1) * fsz], x4_d[z0i, g])
            slabs[z0i] = tl

        def do_slab(z0i, split_out=False):
            v = slabs[z0i].rearrange("p (g t n) -> p g t n", g=4, t=TL)
            ob = outp_pool.tile([128, TP * NCOLS], BF16, tag="out")
            obv = ob.rearrange("p (t n) -> p t n", t=TP)
            for tp in range(TP):
                ps = mpsum_pool.tile([128, NCOLS], F32, tag="mps")
                for j in range(NTAP):
                    dt_, g = divmod(j, 4)
                    rhs = v[0:KROWS, g, tp + dt_]
                    lhsT = wt_s[0:KROWS, j * 128 : (j + 1) * 128]
                    nc.tensor.matmul(
                        ps[:, :], lhsT, rhs, start=(j == 0), stop=(j == NTAP - 1)
                    )
                nc.vector.tensor_scalar_add(
                    ob[:, tp * NCOLS : (tp + 1) * NCOLS], ps[:, :], bias_s[:, 0:1]
                )
                if split_out and tp % 3 == 2:
                    # drain the tail: ship each completed third immediately
                    nc.scalar.dma_start(
                        out_d[z0i, :, tp - 2 : tp + 1], obv[:, tp - 2 : tp + 1]
                    )
            if not split_out:
                nc.scalar.dma_start(out_d[z0i], obv)

        def body():
            slabs.clear()
            prep_slab(0, split_first=True)
            prep_slab(1)
            for z0i in range(NZ0):
                if z0i + 2 < NZ0:
                    prep_slab(z0i + 2)
                do_slab(z0i, split_out=(z0i == NZ0 - 1))
                del slabs[z0i]

        if REPS > 1:
            with tc.For_i(0, REPS, 1):
                body()
        else:
            body()

    nc.compile()
    return nc


def _host_weights(kern):
    """wt[(dt,g), (zl,hl,wl,cc), (z',h',w',co)] banded block weights."""
    wt = np.zeros((KT, 4, 4, 4, 4, 2, 2, 2, 2, COUT), np.float32)
    # idx: [dt, g, zl, hl, wl, cc, z', h', w', co]
    kr = kern.reshape(KT, KZ, KH, KW, 4, 2, COUT)  # c = 2g + cc
    for dz in range(KZ):
        for dh in range(KH):
            for dw in range(KW):
                for zp in range(2):
                    for hp in range(2):
                        for wp in range(2):
                            wt[:, :, zp + dz, hp + dh, wp + dw, :, zp, hp, wp, :] = kr[
                                :, dz, dh, dw
                            ]
    return wt.reshape(NTAP, KROWS, 128).astype(ml_dtypes.bfloat16)


def _host_x4(xc):
    """Pack a core's x slice [TL,Z,H,W,CIN] f32 into
    x4[z0i, g, (zl hl wl cc), (t, h0, w0)] bf16."""
    xb = xc.astype(ml_dtypes.bfloat16).astype(np.float32)
    # windows[t, z0, h0, w0, c, zl, hl, wl] with stride-2 origins
    win = np.lib.stride_tricks.sliding_window_view(xb, (4, 4, 4), axis=(1, 2, 3))
    win = win[:, ::2, ::2, ::2]  # [TL, 9, 19, 19, 8, 4, 4, 4]
    # -> [z0, zl, hl, wl, c, t, h0, w0]
    win = win.transpose(1, 5, 6, 7, 4, 0, 2, 3)
    # split c -> (g, cc) and move g in front of the row dims
    win = win.reshape(NZ0, 4, 4, 4, 4, 2, TL, NG, NG).transpose(
        0, 4, 1, 2, 3, 5, 6, 7, 8
    )
    x4 = win.reshape(NZ0, 4, KROWS, TL * NCOLS)
    return np.ascontiguousarray(x4).astype(ml_dtypes.bfloat16)


_WT_CACHE = {}


def _core_inputs(x, kern, bias, core):
    key = kern.tobytes()
    if _WT_CACHE.get("key") != key:
        _WT_CACHE["key"] = key
        _WT_CACHE["wt"] = _host_weights(kern)
    wt = _WT_CACHE["wt"]
    bias128 = np.tile(bias, 8).reshape(128, 1).astype(np.float32)
    b, th = divmod(core, 2)
    return {
        "x4": _host_x4(x[b, 9 * th : 9 * th + TL]),
        "wt": wt,
        "bias128": bias128,
    }


def _assemble_core_output(outs, core):
    # out [NZ0, (2z' 2h' 2w' 16co), TP, (19h0, 19w0)] bf16
    a = np.asarray(outs["out"]).reshape(NZ0, 2, 2, 2, COUT, TP, NG, NG)
    # -> [t', (z0 z'), (h0 h'), (w0 w'), co]
    a = a.transpose(5, 0, 1, 6, 2, 7, 3, 4)
    a = a.reshape(TP, ZP, HP, WP, COUT)
    return a.astype(np.float32)


def kernel(x, kernel, bias):
    global LAST_RESULTS
    x = np.asarray(x, np.float32)
    kern = np.asarray(kernel, np.float32)
    bias = np.asarray(bias, np.float32).reshape(COUT)

    nc = _build_program()

    core_ids = list(range(8))
    in_maps = [_core_inputs(x, kern, bias, core) for core in core_ids]

    res = run_bass_kernel_spmd(nc, in_maps, core_ids)
    LAST_RESULTS = res

    out = np.empty((B, 2 * TP, ZP, HP, WP, COUT), np.float32)
    for core in core_ids:
        b, th = divmod(core, 2)
        out[b, 9 * th : 9 * th + TP] = _assemble_core_output(res.results[core], core)
    return out


# revision 5
# speedup vs baseline: 119.4585x; 1.0884x over previous
"""Trainium2 Bass kernel for 4D conv (VALID, stride 1, channels-last).

x: [4, 20, 20, 40, 40, 8] f32, kernel: [3, 3, 3, 3, 8, 16], bias: [...,16]
out: [4, 18, 18, 38, 38, 16] f32

Polyphase / space-to-depth scheme (8 NeuronCores, SPMD):
  - Shard (batch 4) x (T-halves 2) -> 8 shards; core computes out t' 9-slab.
  - Outputs tiled in 2x2x2 blocks over (z',h',w'): M = 128 = (2z' 2h' 2w' 16co).
  - Contraction K = 128 = (4zl x 4hl x 4wl x 2cc): the full 4x4x4 input
    window of a 2x2x2 output block, for a 2-channel group. All three
    spatial kernel dims are folded into K; PSUM accumulates only
    12 taps = (3 dt) x (4 channel-groups).
  - N = 361 = the 19x19 (h0,w0) block grid (exact, no padding).
  - Host pre-packs x into x4[z0, (g, rows, t, h0, w0)] bf16 (8x window
    duplication; ~36.6 MB per core) and the 12 banded weight matrices.
    Output written bf16 [z0, 128, (tp, h0, w0)]; host rearranges+casts.
  - One DMA in per z-slab (4.07 MB), one DMA out per z-slab (832 KB).
  PE: 972 matmuls x 361 cycles ~= 146 us; ~44 MB DMA ~= 134 us (overlapped).
"""

import sys

if "/opt/trn_rl_repo" not in sys.path:
    sys.path.insert(0, "/opt/trn_rl_repo")

from contextlib import ExitStack

import ml_dtypes
import numpy as np

import concourse.bass as bass
import concourse.tile as tile
from concourse import bacc, mybir
from concourse.bass_utils import run_bass_kernel_spmd

F32 = mybir.dt.float32
BF16 = mybir.dt.bfloat16

# Problem geometry (hardcoded)
B, T, Z, H, W, CIN = 4, 20, 20, 40, 40, 8
KT = KZ = KH = KW = 3
COUT = 16
TP = 9  # output t' per core (T' = 18 split across 2 cores)
TL = TP + KT - 1  # input t planes per core = 11
ZP, HP, WP = Z - 2, H - 2, W - 2  # 18, 38, 38
NZ0 = ZP // 2  # 9 z-block origins (z0 = 2*z0i)
NG = 19  # h0/w0 block grid (2*19 = 38 outputs, exact)
NCOLS = NG * NG  # 361 columns per matmul
NTAP = KT * 4  # 12 taps: (dt, channel-group)
KROWS = 128  # (zl4, hl4, wl4, cc2)

LAST_RESULTS = None
REPS = 1


def _build_program():
    nc = bacc.Bacc("TRN2", target_bir_lowering=False, debug=False, num_devices=8)

    x4_d = nc.dram_tensor(
        "x4", [NZ0, 4, KROWS, TL, NCOLS], BF16, kind="ExternalInput"
    ).ap()
    # slabs 1..8: lower half (zl in {0,1}) duplicates the previous slab's
    # upper half, so only the upper half is fetched from DRAM
    x4b_d = nc.dram_tensor(
        "x4b", [NZ0 - 1, 4, KROWS // 2, TL, NCOLS], BF16, kind="ExternalInput"
    ).ap()
    wt_d = nc.dram_tensor("wt", [NTAP, KROWS, 128], BF16, kind="ExternalInput").ap()
    bias_d = nc.dram_tensor("bias128", [128, 1], F32, kind="ExternalInput").ap()
    out_d = nc.dram_tensor(
        "out", [NZ0, 128, TP, NCOLS], BF16, kind="ExternalOutput"
    ).ap()

    with ExitStack() as ctx:
        tc = ctx.enter_context(tile.TileContext(nc))
        consts = ctx.enter_context(tc.tile_pool(name="consts", bufs=1))
        x_pool = ctx.enter_context(tc.tile_pool(name="xp", bufs=3))
        mpsum_pool = ctx.enter_context(tc.tile_pool(name="mpsum", bufs=6, space="PSUM"))
        outp_pool = ctx.enter_context(tc.tile_pool(name="outp", bufs=2))

        wt_s = consts.tile([KROWS, NTAP * 128], BF16)
        wt_sv = wt_s.rearrange("p (j m) -> p j m", j=NTAP)
        wt_dv = wt_d.rearrange("j p m -> p j m")
        # first two taps land fast so matmuls can start ~2us in
        nc.scalar.dma_start(wt_sv[:, 0:2], wt_dv[:, 0:2])
        nc.scalar.dma_start(wt_sv[:, 2:NTAP], wt_dv[:, 2:NTAP])

        bias_s = consts.tile([128, 1], F32)
        nc.scalar.dma_start(bias_s[:, :], bias_d[:, :])

        slabs = {}

        def prep_slab(z0i, split_first=False):
            tl = x_pool.tile([KROWS, 4 * TL * NCOLS], BF16, tag="x", name=f"x_{z0i}")
            fsz = TL * NCOLS
            if split_first:
                # fetch in consumption order: t-planes 0-2 of each group
                # first so tp=0 matmuls can start ~3us in
                c3 = 3 * NCOLS
                for g in range(4):
                    nc.sync.dma_start(
                        tl[:, g * fsz : g * fsz + c3], x4_d[z0i, g, :, 0:3]
                    )
                for g in range(4):
                    nc.sync.dma_start(
                        tl[:, g * fsz + c3 : (g + 1) * fsz], x4_d[z0i, g, :, 3:TL]
                    )
            else:
                eng = nc.sync if z0i % 2 == 0 else nc.scalar
                oth = nc.scalar if z0i % 2 == 0 else nc.sync
                # lower half (zl 0,1) = previous slab's upper half (on-chip)
                oth.dma_start(tl[0:64, :], slabs[z0i - 1][64:128, :])
                for g in range(4):
                    eng.dma_start(
                        tl[64:128, g * fsz : (g + 1) * fsz], x4b_d[z0i - 1, g]
                    )
            slabs[z0i] = tl

        def do_slab(z0i, split_out=False):
            v = slabs[z0i].rearrange("p (g t n) -> p g t n", g=4, t=TL)
            ob = outp_pool.tile([128, TP * NCOLS], BF16, tag="out")
            obv = ob.rearrange("p (t n) -> p t n", t=TP)
            for tp in range(TP):
                ps = mpsum_pool.tile([128, NCOLS], F32, tag="mps")
                for j in range(NTAP):
                    dt_, g = divmod(j, 4)
                    rhs = v[0:KROWS, g, tp + dt_]
                    lhsT = wt_s[0:KROWS, j * 128 : (j + 1) * 128]
                    nc.tensor.matmul(
                        ps[:, :], lhsT, rhs, start=(j == 0), stop=(j == NTAP - 1)
                    )
                nc.vector.tensor_scalar_add(
                    ob[:, tp * NCOLS : (tp + 1) * NCOLS], ps[:, :], bias_s[:, 0:1]
                )
                if split_out and tp % 3 == 2:
                    # drain the tail: ship each completed third immediately
                    nc.scalar.dma_start(
                        out_d[z0i, :, tp - 2 : tp + 1], obv[:, tp - 2 : tp + 1]
                    )
            if not split_out:
                nc.scalar.dma_start(out_d[z0i], obv)

        def body():
            slabs.clear()
            prep_slab(0, split_first=True)
            prep_slab(1)
            for z0i in range(NZ0):
                if z0i + 2 < NZ0:
                    prep_slab(z0i + 2)
                do_slab(z0i, split_out=(z0i == NZ0 - 1))
                del slabs[z0i]

        if REPS > 1:
            with tc.For_i(0, REPS, 1):
                body()
        else:
            body()

    nc.compile()
    return nc


def _host_weights(kern):
    """wt[(dt,g), (zl,hl,wl,cc), (z',h',w',co)] banded block weights."""
    wt = np.zeros((KT, 4, 4, 4, 4, 2, 2, 2, 2, COUT), np.float32)
    # idx: [dt, g, zl, hl, wl, cc, z', h', w', co]
    kr = kern.reshape(KT, KZ, KH, KW, 4, 2, COUT)  # c = 2g + cc
    for dz in range(KZ):
        for dh in range(KH):
            for dw in range(KW):
                for zp in range(2):
                    for hp in range(2):
                        for wp in range(2):
                            wt[:, :, zp + dz, hp + dh, wp + dw, :, zp, hp, wp, :] = kr[
                                :, dz, dh, dw
                            ]
    return wt.reshape(NTAP, KROWS, 128).astype(ml_dtypes.bfloat16)


def _host_x4(xc):
    """Pack a core's x slice [TL,Z,H,W,CIN] f32 into
    x4[z0i, g, (zl hl wl cc), (t, h0, w0)] bf16."""
    xb = xc.astype(ml_dtypes.bfloat16).astype(np.float32)
    # windows[t, z0, h0, w0, c, zl, hl, wl] with stride-2 origins
    win = np.lib.stride_tricks.sliding_window_view(xb, (4, 4, 4), axis=(1, 2, 3))
    win = win[:, ::2, ::2, ::2]  # [TL, 9, 19, 19, 8, 4, 4, 4]
    # -> [z0, zl, hl, wl, c, t, h0, w0]
    win = win.transpose(1, 5, 6, 7, 4, 0, 2, 3)
    # split c -> (g, cc) and move g in front of the row dims
    win = win.reshape(NZ0, 4, 4, 4, 4, 2, TL, NG, NG).transpose(
        0, 4, 1, 2, 3, 5, 6, 7, 8
    )
    x4 = win.reshape(NZ0, 4, KROWS, TL * NCOLS)
    return np.ascontiguousarray(x4).astype(ml_dtypes.bfloat16)


_WT_CACHE = {}


def _core_inputs(x, kern, bias, core):
    key = kern.tobytes()
    if _WT_CACHE.get("key") != key:
        _WT_CACHE["key"] = key
        _WT_CACHE["wt"] = _host_weights(kern)
    wt = _WT_CACHE["wt"]
    bias128 = np.tile(bias, 8).reshape(128, 1).astype(np.float32)
    b, th = divmod(core, 2)
    x4 = _host_x4(x[b, 9 * th : 9 * th + TL])
    return {
        "x4": x4,
        "x4b": np.ascontiguousarray(x4[1:, :, 64:]),
        "wt": wt,
        "bias128": bias128,
    }


def _assemble_core_output(outs, core):
    # out [NZ0, (2z' 2h' 2w' 16co), TP, (19h0, 19w0)] bf16
    a = np.asarray(outs["out"]).reshape(NZ0, 2, 2, 2, COUT, TP, NG, NG)
    # -> [t', (z0 z'), (h0 h'), (w0 w'), co]
    a = a.transpose(5, 0, 1, 6, 2, 7, 3, 4)
    a = a.reshape(TP, ZP, HP, WP, COUT)
    return a.astype(np.float32)


def kernel(x, kernel, bias):
    global LAST_RESULTS
    x = np.asarray(x, np.float32)
    kern = np.asarray(kernel, np.float32)
    bias = np.asarray(bias, np.float32).reshape(COUT)

    nc = _build_program()

    core_ids = list(range(8))
    in_maps = [_core_inputs(x, kern, bias, core) for core in core_ids]

    res = run_bass_kernel_spmd(nc, in_maps, core_ids)
    LAST_RESULTS = res

    out = np.empty((B, 2 * TP, ZP, HP, WP, COUT), np.float32)
    for core in core_ids:
        b, th = divmod(core, 2)
        out[b, 9 * th : 9 * th + TP] = _assemble_core_output(res.results[core], core)
    return out
